# revision 18
# baseline (speedup 1.0000x reference)
"""CrossLayer (DCN-v2 style) Trainium2 kernel.

Computes  out = x0 * (xl . W)[:, None] + b + xl   for x0, xl [16384, 4096],
W, b [4096] fp32 — data-parallel over 8 NeuronCores (2048 rows each,
W/b replicated).

Per-core per 128-row tile (rows on partitions, d on the free axis):
  1. DVE  scalar_tensor_tensor: t = xl*Wb (discarded), accum s = row-sum [128,1]
  2. DVE  tensor_add:           u = xl + b_bcast
  3. DVE  scalar_tensor_tensor: out = (x0 * s) + u
All compute is on the Vector engine (~13.2us/tile, 16 tiles = ~212us),
under the ~290us HBM roofline (96MB+4MB at ~358GB/s) -> DMA-bound.
"""

import numpy as np

import concourse.bass as bass
import concourse.mybir as mybir
from concourse.bass_utils import run_bass_kernel_spmd
from concourse.tile import TileContext

N_CORES = 8
B, D = 16384, 4096
ROWS = B // N_CORES  # rows per core
P = 128
N_TILES = ROWS // P  # 16
FP32 = mybir.dt.float32

_PROGRAM = None
LAST_RESULT = None  # test harness reads .exec_time_ns off this


def _split_multi_waits(nc: bass.Bass) -> None:
    """The staged neuronxcc walrus encodes at most ONE sync-wait per
    instruction ("Too many sync wait commands"); Tile's scheduler emits
    instructions waiting on several semaphores. Hoist the extra waits onto
    same-engine NoOps inserted immediately before — the sequencer blocks on
    each in turn, which is semantically identical."""
    n = 0
    for fn in nc.m.functions:
        for blk in fn.blocks:
            new_insts = []
            for inst in blk.instructions:
                si = inst.sync_info
                waits = list(si.on_wait) if si is not None and si.on_wait else []
                if len(waits) > 1:
                    for w in waits[:-1]:
                        nop = mybir.InstNoOp(
                            name=f"{inst.name}-waitsplit-{n}",
                            engine=inst.engine,
                            ins=[],
                            outs=[],
                            sync_info=mybir.SyncInfo(on_wait=[w], on_update=[]),
                        )
                        new_insts.append(nop)
                        n += 1
                    inst.sync_info = mybir.SyncInfo(
                        on_wait=[waits[-1]], on_update=list(si.on_update or [])
                    )
                new_insts.append(inst)
            blk.instructions = new_insts


def _build_program() -> bass.Bass:
    nc = bass.Bass()
    x0 = nc.declare_dram_parameter("x0", [ROWS, D], FP32, isOutput=False)
    xl = nc.declare_dram_parameter("xl", [ROWS, D], FP32, isOutput=False)
    W = nc.declare_dram_parameter("W", [D], FP32, isOutput=False)
    b = nc.declare_dram_parameter("b", [D], FP32, isOutput=False)
    out = nc.declare_dram_parameter("out", [ROWS, D], FP32, isOutput=True)

    x0_t = x0[:, :].rearrange("(n p) d -> n p d", p=P)
    xl_t = xl[:, :].rearrange("(n p) d -> n p d", p=P)
    out_t = out[:, :].rearrange("(n p) d -> n p d", p=P)
    w_row = W[:].rearrange("(r d) -> r d", r=1)
    b_row = b[:].rearrange("(r d) -> r d", r=1)

    with TileContext(nc) as tc:
        with (
            tc.tile_pool(name="consts", bufs=1) as cpool,
            tc.tile_pool(name="io", bufs=3) as iopool,
            tc.tile_pool(name="work", bufs=2) as wpool,
            # rows pool sits ABOVE io/work on the SBUF stack so its address
            # zone is never reused by the loop tiles — reuse would add a
            # released-zone dep making the first tile loads wait for the
            # whole broadcast chain (~30us DMA stall at kernel start).
            tc.tile_pool(name="rows", bufs=1) as rpool,
            # 8 bufs = all 8 PSUM banks: PE fires the 16 broadcast matmuls
            # back-to-back (stays warm) while ScalarE drains banks behind it.
            tc.tile_pool(name="psum", bufs=8, space="PSUM") as ppool,
        ):
            # Load W/b once as single 16KB rows, then replicate across the 128
            # partitions on-chip: ones[128,1] (x) row via PE into PSUM, ScalarE
            # copies PSUM->SBUF. Avoids 4MB of HBM reads that a step-0
            # broadcast DMA would issue.
            w_b = cpool.tile([P, D], FP32)
            b_b = cpool.tile([P, D], FP32)
            ones = rpool.tile([33, P], FP32)
            # One 16KB/partition tile holds both rows: W on partition 0, b on
            # partition 32 (PE matmul operands must base at partition 0/32/64,
            # and lhsT/rhs bases must match — hence ones spans both).
            rows = rpool.tile([33, D], FP32)
            nc.sync.dma_start(out=rows[0:1, :], in_=w_row)
            nc.sync.dma_start(out=rows[32:33, :], in_=b_row)
            nc.vector.memset(ones[:, :], 1.0)
            MM_N = 512  # one PSUM bank per matmul
            for r, dst in ((0, w_b), (32, b_b)):
                for j in range(D // MM_N):
                    pt = ppool.tile([P, MM_N], FP32, name="pt")
                    cols = slice(j * MM_N, (j + 1) * MM_N)
                    nc.tensor.matmul(
                        pt[:, :], ones[r : r + 1, :], rows[r : r + 1, cols]
                    )
                    nc.scalar.copy(dst[:, cols], pt[:, :])

            _main_loop(nc, iopool, wpool, x0_t, xl_t, out_t, w_b, b_b)
    _split_multi_waits(nc)
    return nc


def _main_loop(nc, iopool, wpool, x0_t, xl_t, out_t, w_b, b_b):
    for i in range(N_TILES):
        xl_s = iopool.tile([P, D], FP32, name="xl_s")
        x0_s = iopool.tile([P, D], FP32, name="x0_s", bufs=2)
        nc.sync.dma_start(out=xl_s[:, :], in_=xl_t[i])
        nc.sync.dma_start(out=x0_s[:, :], in_=x0_t[i])

        t1 = wpool.tile([P, D], FP32, name="t1")
        s = wpool.tile([P, 1], FP32, name="s")
        u = wpool.tile([P, D], FP32, name="u")
        # DVE does only two full-width passes per tile; the other two ride
        # the otherwise-idle GpSimd and ScalarE engines.
        nc.vector.scalar_tensor_tensor(
            out=t1[:, :],
            in0=xl_s[:, :],
            scalar=1.0,
            in1=w_b[:, :],
            op0=mybir.AluOpType.mult,
            op1=mybir.AluOpType.mult,
            accum_out=s[:, :],
        )
        nc.gpsimd.tensor_add(u[:, :], xl_s[:, :], b_b[:, :])
        # ScalarE: t1 <- x0 * s (activation Copy with per-partition scale),
        # overwriting op-A's dead product. Serializes behind op A, which
        # produced s anyway.
        nc.scalar.mul(t1[:, :], x0_s[:, :], s[:, :])
        # Final combine in-place over u; the store reads a work tile, never
        # an io tile — loads must never wait on stores.
        nc.vector.tensor_add(u[:, :], t1[:, :], u[:, :])
        # Stores go out the Activation HWDGE ring so they don't
        # head-of-line block the next tile's loads on the SP ring.
        nc.scalar.dma_start(out=out_t[i], in_=u[:, :])


def kernel(x0, xl, W, b, _trace=False, **trace_kwargs):
    global _PROGRAM, LAST_RESULT
    if _PROGRAM is None:
        _PROGRAM = _build_program()

    x0 = np.ascontiguousarray(np.asarray(x0, dtype=np.float32))
    xl = np.ascontiguousarray(np.asarray(xl, dtype=np.float32))
    W = np.ascontiguousarray(np.asarray(W, dtype=np.float32))
    b = np.ascontiguousarray(np.asarray(b, dtype=np.float32))

    in_maps = [
        {
            "x0": x0[c * ROWS : (c + 1) * ROWS],
            "xl": xl[c * ROWS : (c + 1) * ROWS],
            "W": W,
            "b": b,
        }
        for c in range(N_CORES)
    ]
    res = run_bass_kernel_spmd(
        _PROGRAM, in_maps, list(range(N_CORES)), trace=_trace, **trace_kwargs
    )
    LAST_RESULT = res
    return np.concatenate([r["out"] for r in res.results], axis=0)


# revision 19
# speedup vs baseline: 1.2110x; 1.2110x over previous
"""CrossLayer (DCN-v2 style) Trainium2 kernel.

Computes  out = x0 * (xl . W)[:, None] + b + xl   for x0, xl [16384, 4096],
W, b [4096] fp32 — data-parallel over 8 NeuronCores (2048 rows each,
W/b replicated).

Per-core per 128-row tile (rows on partitions, d on the free axis):
  1. DVE  scalar_tensor_tensor: t = xl*Wb (discarded), accum s = row-sum [128,1]
  2. DVE  tensor_add:           u = xl + b_bcast
  3. DVE  scalar_tensor_tensor: out = (x0 * s) + u
All compute is on the Vector engine (~13.2us/tile, 16 tiles = ~212us),
under the ~290us HBM roofline (96MB+4MB at ~358GB/s) -> DMA-bound.
"""

import numpy as np

import concourse.bass as bass
import concourse.mybir as mybir
from concourse.bass_utils import run_bass_kernel_spmd
from concourse.tile import TileContext

N_CORES = 8
B, D = 16384, 4096
ROWS = B // N_CORES  # rows per core
P = 128
N_TILES = ROWS // P  # 16
FP32 = mybir.dt.float32

_PROGRAM = None
LAST_RESULT = None  # test harness reads .exec_time_ns off this


def _split_multi_waits(nc: bass.Bass) -> None:
    """The staged neuronxcc walrus encodes at most ONE sync-wait per
    instruction ("Too many sync wait commands"); Tile's scheduler emits
    instructions waiting on several semaphores. Hoist the extra waits onto
    same-engine NoOps inserted immediately before — the sequencer blocks on
    each in turn, which is semantically identical."""
    n = 0
    for fn in nc.m.functions:
        for blk in fn.blocks:
            new_insts = []
            for inst in blk.instructions:
                si = inst.sync_info
                waits = list(si.on_wait) if si is not None and si.on_wait else []
                if len(waits) > 1:
                    for w in waits[:-1]:
                        nop = mybir.InstNoOp(
                            name=f"{inst.name}-waitsplit-{n}",
                            engine=inst.engine,
                            ins=[],
                            outs=[],
                            sync_info=mybir.SyncInfo(on_wait=[w], on_update=[]),
                        )
                        new_insts.append(nop)
                        n += 1
                    inst.sync_info = mybir.SyncInfo(
                        on_wait=[waits[-1]], on_update=list(si.on_update or [])
                    )
                new_insts.append(inst)
            blk.instructions = new_insts


def _build_program() -> bass.Bass:
    nc = bass.Bass()
    x0 = nc.declare_dram_parameter("x0", [ROWS, D], FP32, isOutput=False)
    xl = nc.declare_dram_parameter("xl", [ROWS, D], FP32, isOutput=False)
    W = nc.declare_dram_parameter("W", [D], FP32, isOutput=False)
    b = nc.declare_dram_parameter("b", [D], FP32, isOutput=False)
    out = nc.declare_dram_parameter("out", [ROWS, D], FP32, isOutput=True)

    x0_t = x0[:, :].rearrange("(n p) d -> n p d", p=P)
    xl_t = xl[:, :].rearrange("(n p) d -> n p d", p=P)
    out_t = out[:, :].rearrange("(n p) d -> n p d", p=P)
    w_row = W[:].rearrange("(r d) -> r d", r=1)
    b_row = b[:].rearrange("(r d) -> r d", r=1)

    with TileContext(nc) as tc:
        with (
            tc.tile_pool(name="consts", bufs=1) as cpool,
            tc.tile_pool(name="io", bufs=3) as iopool,
            tc.tile_pool(name="work", bufs=2) as wpool,
            # rows pool sits ABOVE io/work on the SBUF stack so its address
            # zone is never reused by the loop tiles — reuse would add a
            # released-zone dep making the first tile loads wait for the
            # whole broadcast chain (~30us DMA stall at kernel start).
            tc.tile_pool(name="rows", bufs=1) as rpool,
            # 8 bufs = all 8 PSUM banks: PE fires the 16 broadcast matmuls
            # back-to-back (stays warm) while ScalarE drains banks behind it.
            tc.tile_pool(name="psum", bufs=8, space="PSUM") as ppool,
        ):
            # Load W/b once as single 16KB rows, then replicate across the 128
            # partitions on-chip: ones[128,1] (x) row via PE into PSUM, ScalarE
            # copies PSUM->SBUF. Avoids 4MB of HBM reads that a step-0
            # broadcast DMA would issue.
            w_b = cpool.tile([P, D], FP32)
            b_b = cpool.tile([P, D], FP32)
            ones = rpool.tile([33, P], FP32)
            # One 16KB/partition tile holds both rows: W on partition 0, b on
            # partition 32 (PE matmul operands must base at partition 0/32/64,
            # and lhsT/rhs bases must match — hence ones spans both).
            rows = rpool.tile([33, D], FP32)
            nc.sync.dma_start(out=rows[0:1, :], in_=w_row)
            nc.sync.dma_start(out=rows[32:33, :], in_=b_row)
            nc.vector.memset(ones[:, :], 1.0)
            MM_N = 512  # one PSUM bank per matmul
            for r, dst in ((0, w_b), (32, b_b)):
                for j in range(D // MM_N):
                    pt = ppool.tile([P, MM_N], FP32, name="pt")
                    cols = slice(j * MM_N, (j + 1) * MM_N)
                    nc.tensor.matmul(
                        pt[:, :], ones[r : r + 1, :], rows[r : r + 1, cols]
                    )
                    nc.scalar.copy(dst[:, cols], pt[:, :])

            _main_loop(nc, iopool, wpool, x0_t, xl_t, out_t, w_b, b_b)
    _split_multi_waits(nc)
    return nc


def _main_loop(nc, iopool, wpool, x0_t, xl_t, out_t, w_b, b_b):
    # All three full-width passes stay on DVE: GpSimd shares its SBUF port
    # with DVE (offloading there slowed BOTH engines ~70%, measured), and a
    # ScalarE x0*s pass saves nothing since STT fuses the scale for free.
    for i in range(N_TILES):
        xl_s = iopool.tile([P, D], FP32, name="xl_s")
        x0_s = iopool.tile([P, D], FP32, name="x0_s")
        nc.sync.dma_start(out=xl_s[:, :], in_=xl_t[i])
        nc.sync.dma_start(out=x0_s[:, :], in_=x0_t[i])

        t1 = wpool.tile([P, D], FP32, name="t1")
        s = wpool.tile([P, 1], FP32, name="s")
        # u single-buffered: DVE runs in order, so u(i+1)'s producer never
        # overtakes u(i)'s consumer; saves 16KB/partition of SBUF.
        u = wpool.tile([P, D], FP32, name="u", bufs=1)
        nc.vector.scalar_tensor_tensor(
            out=t1[:, :],
            in0=xl_s[:, :],
            scalar=1.0,
            in1=w_b[:, :],
            op0=mybir.AluOpType.mult,
            op1=mybir.AluOpType.mult,
            accum_out=s[:, :],
        )
        nc.vector.tensor_add(u[:, :], xl_s[:, :], b_b[:, :])
        # Result lands back in t1 (its op-A contents are dead once s is
        # out) so the store reads a work tile, never an io tile — loads
        # must never wait on stores.
        nc.vector.scalar_tensor_tensor(
            out=t1[:, :],
            in0=x0_s[:, :],
            scalar=s[:, :],
            in1=u[:, :],
            op0=mybir.AluOpType.mult,
            op1=mybir.AluOpType.add,
        )
        # Stores go out the Activation HWDGE ring so they don't
        # head-of-line block the next tile's loads on the SP ring.
        nc.scalar.dma_start(out=out_t[i], in_=t1[:, :])


def kernel(x0, xl, W, b, _trace=False, **trace_kwargs):
    global _PROGRAM, LAST_RESULT
    if _PROGRAM is None:
        _PROGRAM = _build_program()

    x0 = np.ascontiguousarray(np.asarray(x0, dtype=np.float32))
    xl = np.ascontiguousarray(np.asarray(xl, dtype=np.float32))
    W = np.ascontiguousarray(np.asarray(W, dtype=np.float32))
    b = np.ascontiguousarray(np.asarray(b, dtype=np.float32))

    in_maps = [
        {
            "x0": x0[c * ROWS : (c + 1) * ROWS],
            "xl": xl[c * ROWS : (c + 1) * ROWS],
            "W": W,
            "b": b,
        }
        for c in range(N_CORES)
    ]
    res = run_bass_kernel_spmd(
        _PROGRAM, in_maps, list(range(N_CORES)), trace=_trace, **trace_kwargs
    )
    LAST_RESULT = res
    return np.concatenate([r["out"] for r in res.results], axis=0)
